# revision 5
# baseline (speedup 1.0000x reference)
"""DSAttention Trainium2 kernel (8 NeuronCores, SPMD).

Sharding: batch (B=2) x head-groups (4 heads each) -> 8 cores.
Core c handles batch b=c//4, heads 4*(c%4) .. 4*(c%4)+3.

Per-core math (feature-major "transposed" layouts so softmax bias/scale land
on partition axes):
  q_t = Wq_p @ hs_b.T          [256, 2048]   (+bq per-partition)
  k_t = Wk_p @ hs_b.T          [256, 2048]   (+bk per-partition)
  v   = hs_b @ Wv_p.T          [2048, 256]   (per k-tile, with a ones column
                                              per head -> softmax denominator)
  s_t[k, q] = k_t.T q_t        per head, one k-tile x all 2048 q at a time
  e_t = exp(s_t * tau/8 + delta_k/8)         (fused ACT exp, N=1024 halves;
                                              no max-subtraction: |logits|<~12)
  ctx_t[65, q] = [v | 1].T @ e_t             accumulated over 16 k-tiles;
                                              row 64 = denominator
  ctx_t[0:64] *= 1/ctx_t[64]                 (PE rank-1 broadcast of d, then
                                              64-lane DVE reciprocal + mul)
  out_partial = ctx.T @ Wo_p.T               [2048, 1024]
Host: out[b] = sum of the 4 head-group partials + bv @ Wo.T + bo
(softmax rows sum to 1, so the v/out biases commute to the host exactly).

All matmuls in float32r (~1.2e-4 input rounding, full PE rate at N>=256).
Phase B is software-pipelined: ctx matmuls for k-tile kt-1 are emitted after
the scores matmuls for kt so the PE queue never drains waiting on ACT.
"""

import sys

for _p in ("/opt/trn_rl_repo", "/opt/pypackages"):
    if _p not in sys.path:
        sys.path.append(_p)

import numpy as np

import concourse.bass as bass
import concourse.tile as tile
from concourse import bacc, mybir
from concourse.bass_utils import run_bass_kernel_spmd

B, L, H = 2, 2048, 1024
NH, HD = 16, 64
NCORES = 8
HPC = 4  # heads per core
FPC = HPC * HD  # 256
NKT = L // 128  # 16 k-tiles
NHC = H // 128  # 8 H-contraction chunks

F32 = mybir.dt.float32
F32R = mybir.dt.float32r

_NC_CACHE = {}

# Dedup consecutive identical LDWEIGHTS in walrus codegen: every fp32r matmul
# self-loads its stationary operand, and consecutive matmuls often share it.
import concourse.bass_utils as _bu

_orig_run_command = _bu.run_command


def _run_command_ldwopt(cmd, *a, **kw):
    if isinstance(cmd, list):
        cmd = [
            "--enable-ldw-opt=true" if c == "--enable-ldw-opt=false" else c
            for c in cmd
        ]
    return _orig_run_command(cmd, *a, **kw)


_bu.run_command = _run_command_ldwopt


def _build_kernel():
    nc = bacc.Bacc(None, target_bir_lowering=False, debug=False)

    hs_t = nc.declare_dram_parameter("hs_t", [H, L], F32, isOutput=False)
    wq_t = nc.declare_dram_parameter("wq_t", [H, FPC], F32, isOutput=False)
    wk_t = nc.declare_dram_parameter("wk_t", [H, FPC], F32, isOutput=False)
    wv_t = nc.declare_dram_parameter("wv_t", [H, FPC], F32, isOutput=False)
    wo_t = nc.declare_dram_parameter("wo_t", [FPC, H], F32, isOutput=False)
    bq2 = nc.declare_dram_parameter("bq2", [128, 2], F32, isOutput=False)
    bk2 = nc.declare_dram_parameter("bk2", [128, 2], F32, isOutput=False)
    tau8 = nc.declare_dram_parameter("tau8", [128, 1], F32, isOutput=False)
    delta8 = nc.declare_dram_parameter("delta8", [128, NKT], F32, isOutput=False)
    out = nc.declare_dram_parameter("out", [L, H], F32, isOutput=True)

    with tile.TileContext(nc) as tc:
        with (
            tc.tile_pool(name="persist", bufs=1) as persist,
            tc.tile_pool(name="hsw", bufs=1) as hsw,
            # PSUM: "sc" 2 x [128,1024] slots (4 banks) + "ctx" 4 x 2KB (4 banks)
            tc.tile_pool(name="sc_ps", bufs=2, space="PSUM") as sc_ps,
            tc.tile_pool(name="ctx_ps", bufs=4, space="PSUM") as ctx_ps,
            tc.tile_pool(name="work", bufs=4) as work,
            tc.tile_pool(name="dscratch", bufs=2, space="DRAM") as dscratch,
        ):
            # ---- input loads -------------------------------------------------
            hs_sb = []
            for c in range(NHC):
                t = hsw.tile([128, L], F32R, tag=f"hs{c}", name=f"hs{c}")
                nc.sync.dma_start(out=t[:], in_=hs_t[c * 128 : (c + 1) * 128, :].bitcast(F32R))
                hs_sb.append(t)
            w_sb = {}
            for name, w in (("q", wq_t), ("k", wk_t), ("v", wv_t)):
                tiles = []
                for c in range(NHC):
                    t = hsw.tile([128, FPC], F32R, tag=f"w{name}{c}", name=f"w{name}{c}")
                    nc.scalar.dma_start(out=t[:], in_=w[c * 128 : (c + 1) * 128, :].bitcast(F32R))
                    tiles.append(t)
                w_sb[name] = tiles
            wo_sb = []
            for c in range(2):
                t = persist.tile([128, H], F32R, tag=f"wo{c}", name=f"wo{c}")
                nc.scalar.dma_start(out=t[:], in_=wo_t[c * 128 : (c + 1) * 128, :].bitcast(F32R))
                wo_sb.append(t)
            bq_sb = persist.tile([128, 2], F32, tag="bq")
            nc.sync.dma_start(out=bq_sb[:], in_=bq2[:])
            bk_sb = persist.tile([128, 2], F32, tag="bk")
            nc.sync.dma_start(out=bk_sb[:], in_=bk2[:])
            tau_sb = persist.tile([128, 1], F32, tag="tau")
            nc.sync.dma_start(out=tau_sb[:], in_=tau8[:])
            del8_sb = persist.tile([128, NKT], F32, tag="del8")
            nc.sync.dma_start(out=del8_sb[:], in_=delta8[:])
            vones_f = persist.tile([128, HPC], F32, tag="vones_f")
            nc.vector.memset(vones_f[:], 1.0)

            # ---- phase A: projections ---------------------------------------
            q_sb = [persist.tile([128, L], F32R, tag=f"q{hp}", name=f"q{hp}") for hp in range(2)]
            k_sb = [persist.tile([128, L], F32R, tag=f"k{hp}", name=f"k{hp}") for hp in range(2)]
            for dst, wname, bias in ((q_sb, "q", bq_sb), (k_sb, "k", bk_sb)):
                for hp in range(2):
                    ps2 = [
                        sc_ps.tile(
                            [128, 1024], F32, tag=f"sc{half}", name=f"ps_proj{half}", bufs=1
                        )
                        for half in range(2)
                    ]
                    for c in range(NHC):
                        # one stationary load serves all 4 N=512 matmuls
                        for half in range(2):
                            for s2 in range(2):
                                nc.tensor.matmul(
                                    ps2[half][:, s2 * 512 : (s2 + 1) * 512],
                                    w_sb[wname][c][:, hp * 128 : (hp + 1) * 128],
                                    hs_sb[c][:, half * 1024 + s2 * 512 : half * 1024 + (s2 + 1) * 512],
                                    start=(c == 0),
                                    stop=(c == NHC - 1),
                                )
                    for half in range(2):
                        nc.vector.tensor_scalar_add(
                            dst[hp][:, half * 1024 : half * 1024 + 1024],
                            ps2[half][:],
                            bias[:, hp : hp + 1],
                        )

            # v: per k-tile [128, 4*65]; head h cols h*65..h*65+63, col h*65+64 = 1
            v_sb = [persist.tile([128, HPC * 65], F32R, tag=f"v{kt}", name=f"v{kt}") for kt in range(NKT)]
            for kt in range(NKT):
                ps = ctx_ps.tile([128, FPC], F32, tag="ctx", name="ps_vproj")
                for c in range(NHC):
                    nc.tensor.matmul(
                        ps[:],
                        hs_sb[c][:, kt * 128 : (kt + 1) * 128],
                        w_sb["v"][c][:],
                        start=(c == 0),
                        stop=(c == NHC - 1),
                    )
                v_view = v_sb[kt][:].rearrange("p (h w) -> p h w", h=HPC)
                nc.vector.tensor_copy(
                    v_view[:, :, 0:HD],
                    ps[:].rearrange("p (h w) -> p h w", h=HPC),
                )
                nc.vector.tensor_copy(v_view[:, :, HD : HD + 1].squeeze(), vones_f[:])

            # ---- phase B (+ phase C interleaved) ----------------------------
            # Structure: head-PAIR x q-half x k-tile. Per k-tile: 2+2 scores
            # matmuls (heads A/B of the pair on partitions 0-63 / 64-127, so
            # they row-tile into disjoint PE row-groups), 2 fused exps
            # (N=1024, one per head, back-to-back on ACT), and 2+2 ctx
            # matmuls. ACT is the bottleneck engine (~2.1-2.4us/iter); all PE
            # stalls (single-buffered per-head scores PSUM) hide under it.
            # PSUM: scA(2 banks) + scB(2) + 4 ctx accumulators (4) = 8 banks.
            ctx_sb = [persist.tile([128, L], F32R, tag=f"ctx{hp}", name=f"ctx{hp}") for hp in range(2)]

            def emit_c_chunk(lts, paired=False):
                for lt in lts:
                    if paired:
                        # 2 psum tiles, one LDW per c-chunk serving both nch
                        pso = [
                            ctx_ps.tile([128, 512], F32, tag="ctx", name=f"ps_o{n}")
                            for n in range(2)
                        ]
                        for c in range(2):
                            for nch in range(2):
                                nc.tensor.matmul(
                                    pso[nch][:],
                                    ctx_sb[c][:, lt * 128 : (lt + 1) * 128],
                                    wo_sb[c][:, nch * 512 : (nch + 1) * 512],
                                    start=(c == 0),
                                    stop=(c == 1),
                                )
                        for nch in range(2):
                            o_sb = work.tile([128, 512], F32, tag="ostage", name="o_sb", bufs=3)
                            nc.vector.tensor_copy(o_sb[:], pso[nch][:])
                            nc.sync.dma_start(
                                out=out[lt * 128 : (lt + 1) * 128, nch * 512 : (nch + 1) * 512],
                                in_=o_sb[:],
                            )
                        continue
                    # serial PSUM use: one pso tile in flight at a time
                    for nch in range(2):
                        pso = ctx_ps.tile([128, 512], F32, tag="ctx", name="ps_o")
                        for c in range(2):
                            nc.tensor.matmul(
                                pso[:],
                                ctx_sb[c][:, lt * 128 : (lt + 1) * 128],
                                wo_sb[c][:, nch * 512 : (nch + 1) * 512],
                                start=(c == 0),
                                stop=(c == 1),
                            )
                        o_sb = work.tile([128, 512], F32, tag="ostage", name="o_sb", bufs=3)
                        nc.vector.tensor_copy(o_sb[:], pso[:])
                        nc.sync.dma_start(
                            out=out[lt * 128 : (lt + 1) * 128, nch * 512 : (nch + 1) * 512],
                            in_=o_sb[:],
                        )

            for hp in range(2):
                for half in range(2):
                    qoff = half * 1024
                    # 4 ctx accumulators: [head-in-pair][g2]
                    ctx2 = [
                        [
                            ctx_ps.tile(
                                [65, 512], F32, tag="ctx", name=f"ctx_p{hp}f{half}h{hh}g{g2}"
                            )
                            for g2 in range(2)
                        ]
                        for hh in range(2)
                    ]

                    def emit_ctx(kt0, es, hp=hp, ctx2=ctx2):
                        for hh in range(2):
                            h = hp * 2 + hh
                            for g2 in range(2):
                                nc.tensor.matmul(
                                    ctx2[hh][g2][:],
                                    v_sb[kt0][:, h * 65 : (h + 1) * 65],
                                    es[hh][:, g2 * 512 : (g2 + 1) * 512],
                                    start=(kt0 == 0),
                                    stop=(kt0 == NKT - 1),
                                )

                    prev = None  # (kt, [e_A, e_B])
                    for kt in range(NKT):
                        pss = []
                        for hh in range(2):
                            psS = sc_ps.tile(
                                [128, 1024], F32, tag=f"sc{hh}", name=f"ps_s{hh}", bufs=1
                            )
                            for s2 in range(2):
                                nc.tensor.matmul(
                                    psS[:, s2 * 512 : (s2 + 1) * 512],
                                    k_sb[hp][hh * HD : (hh + 1) * HD, kt * 128 : (kt + 1) * 128],
                                    q_sb[hp][hh * HD : (hh + 1) * HD, qoff + s2 * 512 : qoff + (s2 + 1) * 512],
                                    start=True,
                                    stop=True,
                                )
                            pss.append(psS)
                        if prev is not None:
                            emit_ctx(*prev)
                        es = []
                        for hh in range(2):
                            e_t = work.tile(
                                [128, 1024], F32R, tag=f"e{hh}", name=f"e_t{hh}", bufs=2
                            )
                            nc.scalar.activation(
                                e_t[:],
                                pss[hh][:],
                                mybir.ActivationFunctionType.Exp,
                                bias=del8_sb[:, kt : kt + 1],
                                scale=tau_sb[:],
                            )
                            es.append(e_t)
                        prev = (kt, es)
                    emit_ctx(*prev)

                    # normalize ctx[0:64] / ctx[64]: drain PSUM -> SBUF at once
                    # (frees accumulator banks), then broadcast the denominator
                    # row via DRAM-bounce DMA and divide on DVE — no PE/PSUM.
                    for hh in range(2):
                        raws = []
                        for g2 in range(2):
                            raw = work.tile([65, 512], F32R, tag="raw", name=f"raw{g2}", bufs=4)
                            nc.vector.tensor_copy(raw[:], ctx2[hh][g2][:])
                            raws.append(raw)
                        for g2 in range(2):
                            g_abs = half * 2 + g2
                            d_dram = dscratch.tile([1, 512], F32, tag="ddram", name="d_dram")
                            nc.sync.dma_start(out=d_dram[:], in_=raws[g2][64:65, :].bitcast(F32))
                            d_bc = work.tile([64, 512], F32, tag="dbc", name="d_bc", bufs=2)
                            nc.sync.dma_start(
                                out=d_bc[:],
                                in_=d_dram[0:1, :].to_broadcast([64, 512]),
                            )
                            r_sb = work.tile([64, 512], F32, tag="r", name="r_sb", bufs=2)
                            nc.vector.reciprocal(r_sb[:], d_bc[:])
                            nc.vector.tensor_mul(
                                ctx_sb[hp][hh * HD : (hh + 1) * HD, g_abs * 512 : (g_abs + 1) * 512],
                                raws[g2][0:64, :],
                                r_sb[:],
                            )
            emit_c_chunk(range(0, 16), paired=True)

    nc.compile()
    return nc


def _get_nc():
    if "nc" not in _NC_CACHE:
        _NC_CACHE["nc"] = _build_kernel()
    return _NC_CACHE["nc"]


def _make_in_maps(hidden_states, tau, delta, Wq, Wk, Wv, Wo, bq, bk):
    in_maps = []
    for c in range(NCORES):
        b, hg = divmod(c, HPC)
        fs = slice(hg * FPC, (hg + 1) * FPC)
        in_maps.append(
            {
                "hs_t": np.ascontiguousarray(hidden_states[b].T),
                "wq_t": np.ascontiguousarray(Wq[fs, :].T),
                "wk_t": np.ascontiguousarray(Wk[fs, :].T),
                "wv_t": np.ascontiguousarray(Wv[fs, :].T),
                "wo_t": np.ascontiguousarray(Wo[:, fs].T),
                "bq2": np.ascontiguousarray(bq[fs].reshape(2, 128).T),
                "bk2": np.ascontiguousarray(bk[fs].reshape(2, 128).T),
                "tau8": np.full((128, 1), tau[b, 0] / 8.0, dtype=np.float32),
                "delta8": np.ascontiguousarray((delta[b] / 8.0).reshape(NKT, 128).T),
            }
        )
    return in_maps


def kernel(hidden_states, tau, delta, Wq, bq, Wk, bk, Wv, bv, Wo, bo, _trace=False):
    hidden_states = np.asarray(hidden_states, dtype=np.float32)
    tau = np.asarray(tau, dtype=np.float32)
    delta = np.asarray(delta, dtype=np.float32)
    Wq = np.asarray(Wq, dtype=np.float32)
    Wk = np.asarray(Wk, dtype=np.float32)
    Wv = np.asarray(Wv, dtype=np.float32)
    Wo = np.asarray(Wo, dtype=np.float32)
    bq = np.asarray(bq, dtype=np.float32)
    bk = np.asarray(bk, dtype=np.float32)
    bv = np.asarray(bv, dtype=np.float32)
    bo = np.asarray(bo, dtype=np.float32)

    nc = _get_nc()
    in_maps = _make_in_maps(hidden_states, tau, delta, Wq, Wk, Wv, Wo, bq, bk)
    res = run_bass_kernel_spmd(nc, in_maps, list(range(NCORES)), trace=_trace)

    out = np.zeros((B, L, H), dtype=np.float32)
    for c in range(NCORES):
        out[c // HPC] += res.results[c]["out"]
    # v/out-proj biases commute through softmax-normalized attention exactly
    out += bv @ Wo.T + bo
    if _trace:
        kernel._last_exec_time_ns = res.exec_time_ns
        kernel._last_profile_json = res.profile_json
    return out



# revision 12
# speedup vs baseline: 1.1865x; 1.1865x over previous
"""DSAttention Trainium2 kernel (8 NeuronCores, SPMD).

Sharding: batch (B=2) x head-groups (4 heads each) -> 8 cores.
Core c handles batch b=c//4, heads 4*(c%4) .. 4*(c%4)+3.

Per-core math (feature-major "transposed" layouts so softmax bias/scale land
on partition axes):
  q_t = Wq_p @ hs_b.T          [256, 2048]   (+bq per-partition)
  k_t = Wk_p @ hs_b.T          [256, 2048]   (+bk per-partition)
  v   = hs_b @ Wv_p.T          [2048, 256]   (per k-tile, with a ones column
                                              per head -> softmax denominator)
Phase B is organized as 16 units = (q-quarter 0..3, head-pair 0..1); each
unit's k-tile loop does, per kt:
  scores: head A -> psS[:, 0:512], head B -> psS[:, 512:1024]  (K=64 matmuls
          on PE row-groups 0-63 / 64-127, auto tile_position)
  e = exp(psS * tau/8 + delta_k/8)  -- ONE [128,1024] fused ACT exp per kt;
          ACT is the bottleneck engine (~1.2us/iter), everything else hides
  ctx_h[65, 512] += [v_h | 1].T @ e_half     accumulated over 16 k-tiles
Then ctx[0:64] /= ctx[64] (DRAM-bounce broadcast + DVE reciprocal/mul).
Out-projection chunks are interleaved into later units' k-loops (their dense
K=128 matmuls double as HAM keepalive); a dedicated filler accumulator covers
units that have no out-proj work yet (K=64 scores matmuls do not register in
the HAM activity monitor, and ctx M=65 alone sits below the throttle-hold
threshold -> the whole phase would pin at 1.2 GHz without them).
Host: out[b] = sum of the 4 head-group partials + bv @ Wo.T + bo
(softmax rows sum to 1, so the v/out biases commute to the host exactly).

All matmuls in float32r (~1.2e-4 input rounding, full PE rate at N>=256).
"""

import sys

for _p in ("/opt/trn_rl_repo", "/opt/pypackages"):
    if _p not in sys.path:
        sys.path.append(_p)

import numpy as np

import concourse.bass as bass
import concourse.tile as tile
from concourse import bacc, mybir
from concourse.bass_utils import run_bass_kernel_spmd

B, L, H = 2, 2048, 1024
NH, HD = 16, 64
NCORES = 8
HPC = 4  # heads per core
FPC = HPC * HD  # 256
NKT = L // 128  # 16 k-tiles
NHC = H // 128  # 8 H-contraction chunks

F32 = mybir.dt.float32
F32R = mybir.dt.float32r

_NC_CACHE = {}

# Dedup consecutive identical LDWEIGHTS in walrus codegen: every fp32r matmul
# self-loads its stationary operand, and consecutive matmuls often share it.
import concourse.bass_utils as _bu

_orig_run_command = _bu.run_command


def _run_command_ldwopt(cmd, *a, **kw):
    if isinstance(cmd, list):
        cmd = [
            "--enable-ldw-opt=true" if c == "--enable-ldw-opt=false" else c
            for c in cmd
        ]
    return _orig_run_command(cmd, *a, **kw)


_bu.run_command = _run_command_ldwopt


def _build_kernel():
    nc = bacc.Bacc(None, target_bir_lowering=False, debug=False)

    hs_t = nc.declare_dram_parameter("hs_t", [H, L], F32, isOutput=False)
    wq_t = nc.declare_dram_parameter("wq_t", [H, FPC], F32, isOutput=False)
    wk_t = nc.declare_dram_parameter("wk_t", [H, FPC], F32, isOutput=False)
    wv_t = nc.declare_dram_parameter("wv_t", [H, FPC], F32, isOutput=False)
    wo_t = nc.declare_dram_parameter("wo_t", [FPC, H], F32, isOutput=False)
    bq2 = nc.declare_dram_parameter("bq2", [128, 2], F32, isOutput=False)
    bk2 = nc.declare_dram_parameter("bk2", [128, 2], F32, isOutput=False)
    tau8 = nc.declare_dram_parameter("tau8", [128, 1], F32, isOutput=False)
    delta8 = nc.declare_dram_parameter("delta8", [128, NKT], F32, isOutput=False)
    out = nc.declare_dram_parameter("out", [L, H], F32, isOutput=True)
    scratch = nc.declare_dram_parameter("scratch", [128, 512], F32, isOutput=True)

    with tile.TileContext(nc) as tc:
        with (
            tc.tile_pool(name="persist", bufs=1) as persist,
            tc.tile_pool(name="hsw", bufs=1) as hsw,
            # PSUM (8 banks): scores [128,1024] x2 (4) + ctx [65,512] x2 (2)
            # + out-proj [128,512] x1 (1) + filler [65,512] x1 (1)
            tc.tile_pool(name="sc_ps", bufs=2, space="PSUM") as sc_ps,
            tc.tile_pool(name="ctx_ps", bufs=2, space="PSUM") as ctx_ps,
            tc.tile_pool(name="work", bufs=4) as work,
            tc.tile_pool(name="dscratch", bufs=2, space="DRAM") as dscratch,
        ):
            # ---- input loads: interleave hs + wq/wk per chunk over 3 queues
            hs_sb = []
            w_sb = {"q": [], "k": [], "v": []}
            for c in range(NHC):
                t = hsw.tile([128, L], F32R, tag=f"hs{c}", name=f"hs{c}")
                nc.sync.dma_start(out=t[:], in_=hs_t[c * 128 : (c + 1) * 128, :].bitcast(F32R))
                hs_sb.append(t)
                for name, w in (("q", wq_t), ("k", wk_t)):
                    wt = hsw.tile([128, FPC], F32R, tag=f"w{name}{c}", name=f"w{name}{c}")
                    nc.scalar.dma_start(out=wt[:], in_=w[c * 128 : (c + 1) * 128, :].bitcast(F32R))
                    w_sb[name].append(wt)
            bq_sb = persist.tile([128, 2], F32, tag="bq")
            nc.scalar.dma_start(out=bq_sb[:], in_=bq2[:])
            bk_sb = persist.tile([128, 2], F32, tag="bk")
            nc.scalar.dma_start(out=bk_sb[:], in_=bk2[:])
            tau_sb = persist.tile([128, 1], F32, tag="tau")
            nc.scalar.dma_start(out=tau_sb[:], in_=tau8[:])
            del8_sb = persist.tile([128, NKT], F32, tag="del8")
            nc.scalar.dma_start(out=del8_sb[:], in_=delta8[:])
            for c in range(NHC):
                wt = hsw.tile([128, FPC], F32R, tag=f"wv{c}", name=f"wv{c}")
                nc.scalar.dma_start(out=wt[:], in_=wv_t[c * 128 : (c + 1) * 128, :].bitcast(F32R))
                w_sb["v"].append(wt)
            wo_sb = []
            for c in range(2):
                t = persist.tile([128, H], F32R, tag=f"wo{c}", name=f"wo{c}")
                nc.scalar.dma_start(out=t[:], in_=wo_t[c * 128 : (c + 1) * 128, :].bitcast(F32R))
                wo_sb.append(t)
            vones_f = persist.tile([128, HPC], F32, tag="vones_f")
            nc.vector.memset(vones_f[:], 1.0)

            # ---- phase A: projections ---------------------------------------
            q_sb = [persist.tile([128, L], F32R, tag=f"q{hp}", name=f"q{hp}") for hp in range(2)]
            k_sb = [persist.tile([128, L], F32R, tag=f"k{hp}", name=f"k{hp}") for hp in range(2)]
            for dst, wname, bias in ((q_sb, "q", bq_sb), (k_sb, "k", bk_sb)):
                for hp in range(2):
                    ps2 = [
                        sc_ps.tile(
                            [128, 1024], F32, tag="sc", name=f"ps_proj{half}", bufs=2
                        )
                        for half in range(2)
                    ]
                    for c in range(NHC):
                        # one stationary load serves all 4 N=512 matmuls
                        for half in range(2):
                            for s2 in range(2):
                                nc.tensor.matmul(
                                    ps2[half][:, s2 * 512 : (s2 + 1) * 512],
                                    w_sb[wname][c][:, hp * 128 : (hp + 1) * 128],
                                    hs_sb[c][:, half * 1024 + s2 * 512 : half * 1024 + (s2 + 1) * 512],
                                    start=(c == 0),
                                    stop=(c == NHC - 1),
                                )
                    for half in range(2):
                        nc.vector.tensor_scalar_add(
                            dst[hp][:, half * 1024 : half * 1024 + 1024],
                            ps2[half][:],
                            bias[:, hp : hp + 1],
                        )

            # v: per k-tile [128, 4*65]; head h cols h*65..h*65+63, col h*65+64 = 1
            v_sb = [persist.tile([128, HPC * 65], F32R, tag=f"v{kt}", name=f"v{kt}") for kt in range(NKT)]
            for kt in range(NKT):
                ps = ctx_ps.tile([128, FPC], F32, tag="ctx2", name="ps_vproj", bufs=2)
                for c in range(NHC):
                    nc.tensor.matmul(
                        ps[:],
                        hs_sb[c][:, kt * 128 : (kt + 1) * 128],
                        w_sb["v"][c][:],
                        start=(c == 0),
                        stop=(c == NHC - 1),
                    )
                v_view = v_sb[kt][:].rearrange("p (h w) -> p h w", h=HPC)
                nc.vector.tensor_copy(
                    v_view[:, :, 0:HD],
                    ps[:].rearrange("p (h w) -> p h w", h=HPC),
                )
                nc.vector.tensor_copy(v_view[:, :, HD : HD + 1].squeeze(), vones_f[:])

            # ---- phase B (+ out-projection interleaved) ---------------------
            ctx_sb = [persist.tile([128, L], F32R, tag=f"ctx{hp}", name=f"ctx{hp}") for hp in range(2)]

            # out-projection chunk-units: (lt, nch) -> 2 matmuls + copy + store
            op_state = {}  # in-flight: (pso, lt, nch, next_c)

            def emit_op_step(lt=None, nch=None):
                """One pipeline step of an out-proj chunk: either advance the
                in-flight chunk or start the given one."""
                if op_state.get("pso") is None:
                    if lt is None:
                        return
                    pso = ctx_ps.tile([128, 512], F32, tag="op", name="ps_o", bufs=1)
                    op_state.update(pso=pso, lt=lt, nch=nch, next_c=0)
                pso, clt, cnch, c = (
                    op_state["pso"], op_state["lt"], op_state["nch"], op_state["next_c"]
                )
                if c < 2:
                    nc.tensor.matmul(
                        pso[:],
                        ctx_sb[c][:, clt * 128 : (clt + 1) * 128],
                        wo_sb[c][:, cnch * 512 : (cnch + 1) * 512],
                        start=(c == 0),
                        stop=(c == 1),
                    )
                    op_state["next_c"] = c + 1
                else:
                    o_sb = work.tile([128, 512], F32, tag="ostage", name="o_sb", bufs=3)
                    nc.vector.tensor_copy(o_sb[:], pso[:])
                    nc.sync.dma_start(
                        out=out[clt * 128 : (clt + 1) * 128, cnch * 512 : (cnch + 1) * 512],
                        in_=o_sb[:],
                    )
                    op_state["pso"] = None
                    if lt is not None:
                        emit_op_step(lt, nch)

            # filler: dedicated accumulator, keeps HAM registered-activity up
            fill_ps = ctx_ps.tile([65, 512], F32, tag="fill", name="fill_ps", bufs=1)
            nfill = [0]

            def emit_filler(v_stat, e_mov):
                nc.tensor.matmul(
                    fill_ps[:],
                    v_stat,
                    e_mov,
                    start=(nfill[0] == 0),
                    stop=False,
                    skip_group_check=True,
                )
                nfill[0] += 1

            # chunk-unit queue per unit index u=2*qq+hp: quarter qq's 8 chunk
            # units spread over units 2qq+2 .. 2qq+3 (4 each); quarter 3 tails.
            op_queue = {u: [] for u in range(8)}
            for qq in range(3):
                chunks = [(4 * qq + i, nch) for i in range(4) for nch in range(2)]
                for j, cu in enumerate(chunks):
                    op_queue[2 * qq + 2 + (j // 4)].append(cu)
            tail_chunks = [(4 * 3 + i, nch) for i in range(4) for nch in range(2)]

            for qq in range(4):
                for hp in range(2):
                    u = 2 * qq + hp
                    qoff = qq * 512
                    ops = list(op_queue[u])
                    ctx2 = [
                        ctx_ps.tile([65, 512], F32, tag="ctx2", name=f"ctx_u{u}h{hh}", bufs=2)
                        for hh in range(2)
                    ]

                    def emit_ctx(kt0, e, hp=hp, ctx2=ctx2, u=u):
                        for hh in range(2):
                            h = hp * 2 + hh
                            nc.tensor.matmul(
                                ctx2[hh][:],
                                v_sb[kt0][:, h * 65 : (h + 1) * 65],
                                e[:, hh * 512 : (hh + 1) * 512],
                                start=(kt0 == 0),
                                stop=(kt0 == NKT - 1),
                            )
                            if hh == 0 and not ops and kt0 % 2 == 0:
                                # no out-proj work this unit: keepalive filler
                                emit_filler(
                                    v_sb[kt0][:, h * 65 : (h + 1) * 65],
                                    e[:, 0:512],
                                )

                    prev = None  # (kt, e)
                    for kt in range(NKT):
                        psS = sc_ps.tile([128, 1024], F32, tag="sc", name="ps_s", bufs=2)
                        for hh in range(2):
                            nc.tensor.matmul(
                                psS[:, hh * 512 : (hh + 1) * 512],
                                k_sb[hp][hh * HD : (hh + 1) * HD, kt * 128 : (kt + 1) * 128],
                                q_sb[hp][hh * HD : (hh + 1) * HD, qoff : qoff + 512],
                                start=True,
                                stop=True,
                            )
                        if prev is not None:
                            emit_ctx(*prev)
                        if kt % 4 == 1 and ops:
                            emit_op_step(*ops.pop(0))  # start/queue a chunk
                        elif kt % 2 == 1:
                            emit_op_step()  # advance in-flight chunk
                        e_t = work.tile([128, 1024], F32R, tag="e", name="e_t", bufs=2)
                        nc.scalar.activation(
                            e_t[:],
                            psS[:],
                            mybir.ActivationFunctionType.Exp,
                            bias=del8_sb[:, kt : kt + 1],
                            scale=tau_sb[:],
                        )
                        prev = (kt, e_t)
                    emit_ctx(*prev)

                    # normalize ctx[0:64] / ctx[64]: drain PSUM -> SBUF
                    # (frees accumulator banks), then broadcast the denominator
                    # row via DRAM-bounce DMA and divide on DVE — no PE/PSUM.
                    raws = []
                    for hh in range(2):
                        raw = work.tile([65, 512], F32R, tag="raw", name=f"raw{hh}", bufs=4)
                        nc.vector.tensor_copy(raw[:], ctx2[hh][:])
                        raws.append(raw)
                    for hh in range(2):
                        d_dram = dscratch.tile([1, 512], F32, tag="ddram", name="d_dram")
                        nc.sync.dma_start(out=d_dram[:], in_=raws[hh][64:65, :].bitcast(F32))
                        d_bc = work.tile([64, 512], F32, tag="dbc", name="d_bc", bufs=2)
                        nc.sync.dma_start(
                            out=d_bc[:],
                            in_=d_dram[0:1, :].to_broadcast([64, 512]),
                        )
                        r_sb = work.tile([64, 512], F32, tag="r", name="r_sb", bufs=2)
                        nc.vector.reciprocal(r_sb[:], d_bc[:])
                        nc.vector.tensor_mul(
                            ctx_sb[hp][hh * HD : (hh + 1) * HD, qoff : qoff + 512],
                            raws[hh][0:64, :],
                            r_sb[:],
                        )

            # drain: remaining chunk steps (quarter 3) + any in-flight chunk
            for lt, nch in tail_chunks:
                emit_op_step(lt, nch)
                emit_op_step()
                emit_op_step()

            # close the filler accumulation group, then read it so DCE keeps
            # the keepalives
            nc.tensor.matmul(
                fill_ps[:],
                v_sb[0][:, 0:65],
                ctx_sb[0][:, 0:512],
                start=(nfill[0] == 0),
                stop=True,
                skip_group_check=True,
            )
            fcopy = work.tile([65, 512], F32, tag="ostage2", name="fcopy", bufs=1)
            nc.vector.tensor_copy(fcopy[:], fill_ps[:])
            nc.sync.dma_start(out=scratch[0:65, :], in_=fcopy[:])

    nc.compile()
    return nc


def _get_nc():
    if "nc" not in _NC_CACHE:
        _NC_CACHE["nc"] = _build_kernel()
    return _NC_CACHE["nc"]


def _make_in_maps(hidden_states, tau, delta, Wq, Wk, Wv, Wo, bq, bk):
    in_maps = []
    for c in range(NCORES):
        b, hg = divmod(c, HPC)
        fs = slice(hg * FPC, (hg + 1) * FPC)
        in_maps.append(
            {
                "hs_t": np.ascontiguousarray(hidden_states[b].T),
                "wq_t": np.ascontiguousarray(Wq[fs, :].T),
                "wk_t": np.ascontiguousarray(Wk[fs, :].T),
                "wv_t": np.ascontiguousarray(Wv[fs, :].T),
                "wo_t": np.ascontiguousarray(Wo[:, fs].T),
                "bq2": np.ascontiguousarray(bq[fs].reshape(2, 128).T),
                "bk2": np.ascontiguousarray(bk[fs].reshape(2, 128).T),
                "tau8": np.full((128, 1), tau[b, 0] / 8.0, dtype=np.float32),
                "delta8": np.ascontiguousarray((delta[b] / 8.0).reshape(NKT, 128).T),
            }
        )
    return in_maps


def kernel(hidden_states, tau, delta, Wq, bq, Wk, bk, Wv, bv, Wo, bo, _trace=False):
    hidden_states = np.asarray(hidden_states, dtype=np.float32)
    tau = np.asarray(tau, dtype=np.float32)
    delta = np.asarray(delta, dtype=np.float32)
    Wq = np.asarray(Wq, dtype=np.float32)
    Wk = np.asarray(Wk, dtype=np.float32)
    Wv = np.asarray(Wv, dtype=np.float32)
    Wo = np.asarray(Wo, dtype=np.float32)
    bq = np.asarray(bq, dtype=np.float32)
    bk = np.asarray(bk, dtype=np.float32)
    bv = np.asarray(bv, dtype=np.float32)
    bo = np.asarray(bo, dtype=np.float32)

    nc = _get_nc()
    in_maps = _make_in_maps(hidden_states, tau, delta, Wq, Wk, Wv, Wo, bq, bk)
    res = run_bass_kernel_spmd(nc, in_maps, list(range(NCORES)), trace=_trace)

    out = np.zeros((B, L, H), dtype=np.float32)
    for c in range(NCORES):
        out[c // HPC] += res.results[c]["out"]
    # v/out-proj biases commute through softmax-normalized attention exactly
    out += bv @ Wo.T + bo
    if _trace:
        kernel._last_exec_time_ns = res.exec_time_ns
        kernel._last_profile_json = res.profile_json
    return out


# revision 15
# speedup vs baseline: 1.2052x; 1.0158x over previous
"""DSAttention Trainium2 kernel (8 NeuronCores, SPMD).

Sharding: batch (B=2) x head-groups (4 heads each) -> 8 cores.
Core c handles batch b=c//4, heads 4*(c%4) .. 4*(c%4)+3.

Per-core math (feature-major "transposed" layouts so softmax bias/scale land
on partition axes):
  q_t = Wq_p @ hs_b.T          [256, 2048]   (+bq per-partition)
  k_t = Wk_p @ hs_b.T          [256, 2048]   (+bk per-partition)
  v   = hs_b @ Wv_p.T          [2048, 256]   (per k-tile, with a ones column
                                              per head -> softmax denominator)
Phase B is organized as 16 units = (q-quarter 0..3, head-pair 0..1); each
unit's k-tile loop does, per kt:
  scores: head A -> psS[:, 0:512], head B -> psS[:, 512:1024]  (K=64 matmuls
          on PE row-groups 0-63 / 64-127, auto tile_position)
  e = exp(psS * tau/8 + delta_k/8)  -- ONE [128,1024] fused ACT exp per kt;
          ACT is the bottleneck engine (~1.2us/iter), everything else hides
  ctx_h[65, 512] += [v_h | 1].T @ e_half     accumulated over 16 k-tiles
Then ctx[0:64] /= ctx[64] (DRAM-bounce broadcast + DVE reciprocal/mul).
Out-projection chunks are interleaved into later units' k-loops (their dense
K=128 matmuls double as HAM keepalive); a dedicated filler accumulator covers
units that have no out-proj work yet (K=64 scores matmuls do not register in
the HAM activity monitor, and ctx M=65 alone sits below the throttle-hold
threshold -> the whole phase would pin at 1.2 GHz without them).
Host: out[b] = sum of the 4 head-group partials + bv @ Wo.T + bo
(softmax rows sum to 1, so the v/out biases commute to the host exactly).

All matmuls in float32r (~1.2e-4 input rounding, full PE rate at N>=256).
"""

import sys

for _p in ("/opt/trn_rl_repo", "/opt/pypackages"):
    if _p not in sys.path:
        sys.path.append(_p)

import numpy as np

import concourse.bass as bass
import concourse.tile as tile
from concourse import bacc, mybir
from concourse.bass_utils import run_bass_kernel_spmd

B, L, H = 2, 2048, 1024
NH, HD = 16, 64
NCORES = 8
HPC = 4  # heads per core
FPC = HPC * HD  # 256
NKT = L // 128  # 16 k-tiles
NHC = H // 128  # 8 H-contraction chunks

F32 = mybir.dt.float32
F32R = mybir.dt.float32r

_NC_CACHE = {}

# Dedup consecutive identical LDWEIGHTS in walrus codegen: every fp32r matmul
# self-loads its stationary operand, and consecutive matmuls often share it.
import concourse.bass_utils as _bu

_orig_run_command = _bu.run_command


def _run_command_ldwopt(cmd, *a, **kw):
    if isinstance(cmd, list):
        cmd = [
            "--enable-ldw-opt=true" if c == "--enable-ldw-opt=false" else c
            for c in cmd
        ]
    return _orig_run_command(cmd, *a, **kw)


_bu.run_command = _run_command_ldwopt


def _build_kernel():
    nc = bacc.Bacc(None, target_bir_lowering=False, debug=False)

    hs_t = nc.declare_dram_parameter("hs_t", [H, L], F32, isOutput=False)
    wq_t = nc.declare_dram_parameter("wq_t", [H, FPC], F32, isOutput=False)
    wk_t = nc.declare_dram_parameter("wk_t", [H, FPC], F32, isOutput=False)
    wv_t = nc.declare_dram_parameter("wv_t", [H, FPC], F32, isOutput=False)
    wo_t = nc.declare_dram_parameter("wo_t", [FPC, H], F32, isOutput=False)
    bq2 = nc.declare_dram_parameter("bq2", [128, 2], F32, isOutput=False)
    bk2 = nc.declare_dram_parameter("bk2", [128, 2], F32, isOutput=False)
    tau8 = nc.declare_dram_parameter("tau8", [128, 1], F32, isOutput=False)
    delta8 = nc.declare_dram_parameter("delta8", [128, NKT], F32, isOutput=False)
    out = nc.declare_dram_parameter("out", [L, H], F32, isOutput=True)
    scratch = nc.declare_dram_parameter("scratch", [128, 512], F32, isOutput=True)

    with tile.TileContext(nc) as tc:
        with (
            tc.tile_pool(name="persist", bufs=1) as persist,
            tc.tile_pool(name="hsw", bufs=1) as hsw,
            # PSUM (8 banks): scores [128,1024] x2 (4) + ctx [65,512] x2 (2)
            # + out-proj [128,512] x1 (1) + filler [65,512] x1 (1)
            tc.tile_pool(name="sc_ps", bufs=2, space="PSUM") as sc_ps,
            tc.tile_pool(name="ctx_ps", bufs=2, space="PSUM") as ctx_ps,
            tc.tile_pool(name="work", bufs=4) as work,
            tc.tile_pool(name="dscratch", bufs=2, space="DRAM") as dscratch,
        ):
            # ---- input loads: per-chunk interleave across both HWDGE queues
            # (hs_c on one, wq/wk/wv_c on the other, alternating) so the two
            # rings split the 12MB roughly evenly and chunk-set c lands ~5us
            # after c-1 -> Q/K projection streams behind the DMA.
            hs_sb = []
            w_sb = {"q": [], "k": [], "v": []}
            queues = [nc.sync, nc.scalar]
            for c in range(NHC):
                qa, qb = queues[c % 2], queues[(c + 1) % 2]
                t = hsw.tile([128, L], F32R, tag=f"hs{c}", name=f"hs{c}")
                qa.dma_start(out=t[:], in_=hs_t[c * 128 : (c + 1) * 128, :].bitcast(F32R))
                hs_sb.append(t)
                for name, w in (("q", wq_t), ("k", wk_t), ("v", wv_t)):
                    wt = hsw.tile([128, FPC], F32R, tag=f"w{name}{c}", name=f"w{name}{c}")
                    qb.dma_start(out=wt[:], in_=w[c * 128 : (c + 1) * 128, :].bitcast(F32R))
                    w_sb[name].append(wt)
                if c == 0:
                    bq_sb = persist.tile([128, 2], F32, tag="bq")
                    qa.dma_start(out=bq_sb[:], in_=bq2[:])
                    bk_sb = persist.tile([128, 2], F32, tag="bk")
                    qa.dma_start(out=bk_sb[:], in_=bk2[:])
                    tau_sb = persist.tile([128, 1], F32, tag="tau")
                    qa.dma_start(out=tau_sb[:], in_=tau8[:])
                    del8_sb = persist.tile([128, NKT], F32, tag="del8")
                    qa.dma_start(out=del8_sb[:], in_=delta8[:])
            wo_sb = []
            for c in range(2):
                t = persist.tile([128, H], F32R, tag=f"wo{c}", name=f"wo{c}")
                nc.scalar.dma_start(out=t[:], in_=wo_t[c * 128 : (c + 1) * 128, :].bitcast(F32R))
                wo_sb.append(t)
            vones_f = persist.tile([128, HPC], F32, tag="vones_f")
            nc.vector.memset(vones_f[:], 1.0)

            # ---- phase A: projections ---------------------------------------
            q_sb = [persist.tile([128, L], F32R, tag=f"q{hp}", name=f"q{hp}") for hp in range(2)]
            k_sb = [persist.tile([128, L], F32R, tag=f"k{hp}", name=f"k{hp}") for hp in range(2)]
            for dst, wname, bias in ((q_sb, "q", bq_sb), (k_sb, "k", bk_sb)):
                for hp in range(2):
                    ps2 = [
                        sc_ps.tile(
                            [128, 1024], F32, tag="sc", name=f"ps_proj{half}", bufs=2
                        )
                        for half in range(2)
                    ]
                    for c in range(NHC):
                        # one stationary load serves all 4 N=512 matmuls
                        for half in range(2):
                            for s2 in range(2):
                                nc.tensor.matmul(
                                    ps2[half][:, s2 * 512 : (s2 + 1) * 512],
                                    w_sb[wname][c][:, hp * 128 : (hp + 1) * 128],
                                    hs_sb[c][:, half * 1024 + s2 * 512 : half * 1024 + (s2 + 1) * 512],
                                    start=(c == 0),
                                    stop=(c == NHC - 1),
                                )
                    for half in range(2):
                        nc.vector.tensor_scalar_add(
                            dst[hp][:, half * 1024 : half * 1024 + 1024],
                            ps2[half][:],
                            bias[:, hp : hp + 1],
                        )

            # v: per k-tile [128, 4*65]; head h cols h*65..h*65+63, col h*65+64 = 1
            v_sb = [persist.tile([128, HPC * 65], F32R, tag=f"v{kt}", name=f"v{kt}") for kt in range(NKT)]
            for kt in range(NKT):
                ps = ctx_ps.tile([128, FPC], F32, tag="ctx2", name="ps_vproj", bufs=2)
                for c in range(NHC):
                    nc.tensor.matmul(
                        ps[:],
                        hs_sb[c][:, kt * 128 : (kt + 1) * 128],
                        w_sb["v"][c][:],
                        start=(c == 0),
                        stop=(c == NHC - 1),
                    )
                v_view = v_sb[kt][:].rearrange("p (h w) -> p h w", h=HPC)
                nc.vector.tensor_copy(
                    v_view[:, :, 0:HD],
                    ps[:].rearrange("p (h w) -> p h w", h=HPC),
                )
                nc.vector.tensor_copy(v_view[:, :, HD : HD + 1].squeeze(), vones_f[:])

            # ---- phase B (+ out-projection interleaved) ---------------------
            ctx_sb = [persist.tile([128, L], F32R, tag=f"ctx{hp}", name=f"ctx{hp}") for hp in range(2)]

            # out-projection chunk-units: (lt, nch) -> 2 matmuls + copy + store
            op_state = {}  # in-flight: (pso, lt, nch, next_c)

            def emit_op_step(lt=None, nch=None):
                """One pipeline step of an out-proj chunk: either advance the
                in-flight chunk or start the given one."""
                if op_state.get("pso") is None:
                    if lt is None:
                        return
                    pso = ctx_ps.tile([128, 512], F32, tag="op", name="ps_o", bufs=1)
                    op_state.update(pso=pso, lt=lt, nch=nch, next_c=0)
                pso, clt, cnch, c = (
                    op_state["pso"], op_state["lt"], op_state["nch"], op_state["next_c"]
                )
                if c < 2:
                    nc.tensor.matmul(
                        pso[:],
                        ctx_sb[c][:, clt * 128 : (clt + 1) * 128],
                        wo_sb[c][:, cnch * 512 : (cnch + 1) * 512],
                        start=(c == 0),
                        stop=(c == 1),
                    )
                    op_state["next_c"] = c + 1
                else:
                    o_sb = work.tile([128, 512], F32, tag="ostage", name="o_sb", bufs=3)
                    nc.vector.tensor_copy(o_sb[:], pso[:])
                    nc.sync.dma_start(
                        out=out[clt * 128 : (clt + 1) * 128, cnch * 512 : (cnch + 1) * 512],
                        in_=o_sb[:],
                    )
                    op_state["pso"] = None
                    if lt is not None:
                        emit_op_step(lt, nch)

            # filler: dedicated accumulator, keeps HAM registered-activity up
            fill_ps = ctx_ps.tile([65, 512], F32, tag="fill", name="fill_ps", bufs=1)
            nfill = [0]

            def emit_filler(v_stat, e_mov):
                nc.tensor.matmul(
                    fill_ps[:],
                    v_stat,
                    e_mov,
                    start=(nfill[0] == 0),
                    stop=False,
                    skip_group_check=True,
                )
                nfill[0] += 1

            # chunk-unit queue per unit index u=2*qq+hp: quarter qq's 8 chunk
            # units spread over units 2qq+2 .. 2qq+3 (4 each); quarter 3 tails.
            op_queue = {u: [] for u in range(8)}
            for qq in range(3):
                chunks = [(4 * qq + i, nch) for i in range(4) for nch in range(2)]
                for j, cu in enumerate(chunks):
                    op_queue[2 * qq + 2 + (j // 4)].append(cu)
            tail_chunks = [(4 * 3 + i, nch) for i in range(4) for nch in range(2)]

            for qq in range(4):
                for hp in range(2):
                    u = 2 * qq + hp
                    qoff = qq * 512
                    ops = list(op_queue[u])
                    ctx2 = [
                        ctx_ps.tile([65, 512], F32, tag="ctx2", name=f"ctx_u{u}h{hh}", bufs=2)
                        for hh in range(2)
                    ]

                    def emit_ctx(kt0, e, hp=hp, ctx2=ctx2):
                        for hh in range(2):
                            h = hp * 2 + hh
                            nc.tensor.matmul(
                                ctx2[hh][:],
                                v_sb[kt0][:, h * 65 : (h + 1) * 65],
                                e[:, hh * 512 : (hh + 1) * 512],
                                start=(kt0 == 0),
                                stop=(kt0 == NKT - 1),
                            )

                    prev = None  # (kt, e)
                    for kt in range(NKT):
                        psS = sc_ps.tile([128, 1024], F32, tag="sc", name="ps_s", bufs=2)
                        for hh in range(2):
                            nc.tensor.matmul(
                                psS[:, hh * 512 : (hh + 1) * 512],
                                k_sb[hp][hh * HD : (hh + 1) * HD, kt * 128 : (kt + 1) * 128],
                                q_sb[hp][hh * HD : (hh + 1) * HD, qoff : qoff + 512],
                                start=True,
                                stop=True,
                            )
                        if prev is not None:
                            emit_ctx(*prev)
                        # one dense K=128 op per iter keeps the HAM activity
                        # monitor above its throttle-hold threshold: an
                        # out-proj pipeline step (kt>=4 so the previous unit's
                        # normalize has drained its DRAM bounce), else a
                        # filler on alternate iters.
                        if kt >= 4 and (op_state.get("pso") is not None or ops):
                            if op_state.get("pso") is not None:
                                emit_op_step()  # advance in-flight chunk
                            else:
                                emit_op_step(*ops.pop(0))  # start a chunk
                        elif prev is not None:
                            h1 = hp * 2 + 1
                            emit_filler(
                                v_sb[prev[0]][:, h1 * 65 : (h1 + 1) * 65],
                                prev[1][:, 512:1024],
                            )
                        e_t = work.tile([128, 1024], F32R, tag="e", name="e_t", bufs=2)
                        nc.scalar.activation(
                            e_t[:],
                            psS[:],
                            mybir.ActivationFunctionType.Exp,
                            bias=del8_sb[:, kt : kt + 1],
                            scale=tau_sb[:],
                        )
                        prev = (kt, e_t)
                    emit_ctx(*prev)

                    # normalize ctx[0:64] / ctx[64]: drain PSUM -> SBUF
                    # (frees accumulator banks), then broadcast the denominator
                    # row via DRAM-bounce DMA and divide on DVE — no PE/PSUM.
                    raws = []
                    for hh in range(2):
                        raw = work.tile([65, 512], F32R, tag="raw", name=f"raw{hh}", bufs=4)
                        nc.vector.tensor_copy(raw[:], ctx2[hh][:])
                        raws.append(raw)
                    for hh in range(2):
                        d_dram = dscratch.tile([1, 512], F32, tag="ddram", name="d_dram")
                        nc.sync.dma_start(out=d_dram[:], in_=raws[hh][64:65, :].bitcast(F32))
                        d_bc = work.tile([64, 512], F32, tag="dbc", name="d_bc", bufs=2)
                        nc.sync.dma_start(
                            out=d_bc[:],
                            in_=d_dram[0:1, :].to_broadcast([64, 512]),
                        )
                        r_sb = work.tile([64, 512], F32, tag="r", name="r_sb", bufs=2)
                        nc.vector.reciprocal(r_sb[:], d_bc[:])
                        nc.vector.tensor_mul(
                            ctx_sb[hp][hh * HD : (hh + 1) * HD, qoff : qoff + 512],
                            raws[hh][0:64, :],
                            r_sb[:],
                        )

            # drain: remaining chunk steps (quarter 3) + any in-flight chunk
            for lt, nch in tail_chunks:
                emit_op_step(lt, nch)
                emit_op_step()
                emit_op_step()

            # close the filler accumulation group, then read it so DCE keeps
            # the keepalives
            nc.tensor.matmul(
                fill_ps[:],
                v_sb[0][:, 0:65],
                ctx_sb[0][:, 0:512],
                start=(nfill[0] == 0),
                stop=True,
                skip_group_check=True,
            )
            fcopy = work.tile([65, 512], F32, tag="ostage2", name="fcopy", bufs=1)
            nc.vector.tensor_copy(fcopy[:], fill_ps[:])
            nc.sync.dma_start(out=scratch[0:65, :], in_=fcopy[:])

    nc.compile()
    return nc


def _get_nc():
    if "nc" not in _NC_CACHE:
        _NC_CACHE["nc"] = _build_kernel()
    return _NC_CACHE["nc"]


def _make_in_maps(hidden_states, tau, delta, Wq, Wk, Wv, Wo, bq, bk):
    in_maps = []
    for c in range(NCORES):
        b, hg = divmod(c, HPC)
        fs = slice(hg * FPC, (hg + 1) * FPC)
        in_maps.append(
            {
                "hs_t": np.ascontiguousarray(hidden_states[b].T),
                "wq_t": np.ascontiguousarray(Wq[fs, :].T),
                "wk_t": np.ascontiguousarray(Wk[fs, :].T),
                "wv_t": np.ascontiguousarray(Wv[fs, :].T),
                "wo_t": np.ascontiguousarray(Wo[:, fs].T),
                "bq2": np.ascontiguousarray(bq[fs].reshape(2, 128).T),
                "bk2": np.ascontiguousarray(bk[fs].reshape(2, 128).T),
                "tau8": np.full((128, 1), tau[b, 0] / 8.0, dtype=np.float32),
                "delta8": np.ascontiguousarray((delta[b] / 8.0).reshape(NKT, 128).T),
            }
        )
    return in_maps


def kernel(hidden_states, tau, delta, Wq, bq, Wk, bk, Wv, bv, Wo, bo, _trace=False):
    hidden_states = np.asarray(hidden_states, dtype=np.float32)
    tau = np.asarray(tau, dtype=np.float32)
    delta = np.asarray(delta, dtype=np.float32)
    Wq = np.asarray(Wq, dtype=np.float32)
    Wk = np.asarray(Wk, dtype=np.float32)
    Wv = np.asarray(Wv, dtype=np.float32)
    Wo = np.asarray(Wo, dtype=np.float32)
    bq = np.asarray(bq, dtype=np.float32)
    bk = np.asarray(bk, dtype=np.float32)
    bv = np.asarray(bv, dtype=np.float32)
    bo = np.asarray(bo, dtype=np.float32)

    nc = _get_nc()
    in_maps = _make_in_maps(hidden_states, tau, delta, Wq, Wk, Wv, Wo, bq, bk)
    res = run_bass_kernel_spmd(nc, in_maps, list(range(NCORES)), trace=_trace)

    out = np.zeros((B, L, H), dtype=np.float32)
    for c in range(NCORES):
        out[c // HPC] += res.results[c]["out"]
    # v/out-proj biases commute through softmax-normalized attention exactly
    out += bv @ Wo.T + bo
    if _trace:
        kernel._last_exec_time_ns = res.exec_time_ns
        kernel._last_profile_json = res.profile_json
    return out
